# revision 23
# baseline (speedup 1.0000x reference)
"""LSTMCell (B=65536, H=512) Bass/Tile kernel for 8 trn2 NeuronCores.

Data-parallel over batch: each core processes 8192 rows, as 16 blocks
of 512 batch columns in a TRANSPOSED [feature, batch] layout:

  host: xT/sT staged as bf16 [512, 8192] per core (batch transposed),
        weights packed so each matmul lhsT is a [128h, 128j] W chunk.
  per 512-batch block:
    zT = xT + sT                   (DVE, 4x [128,512] bf16)
    for each of 16 j-chunks (4 gates x 4 slots):
      psum[128j, 512b] = sum_k W_chunk.T @ zT_chunk   (PE, bf16)
      act = sigmoid/tanh(psum + bias_j)  (ACT, per-partition bias fused)
    prod = i*g; c = f + prod; tc = tanh(c); h = tc*o   (DVE + ACT, [128,2048])
    DMA out c,h as bf16 in [h_dim, batch] layout; host transposes back.

vs the previous version this removes the PE transposes, removes the
[128,2048] fp32 DVE bias-add (the old bottleneck: DVE was 76% busy),
and halves DMA traffic (bf16 I/O).
"""

import os
import sys

if "/opt/trn_rl_repo" not in sys.path:
    sys.path.insert(0, "/opt/trn_rl_repo")

import ml_dtypes
import numpy as np

import concourse.bacc as bacc
import concourse.mybir as mybir
import concourse.tile as tile

N_CORES = 8
B, H = 65536, 512
B_CORE = B // N_CORES  # 8192
NB = 512  # batch columns per block
N_BLOCKS = B_CORE // NB  # 16
F32 = mybir.dt.float32
BF16 = mybir.dt.bfloat16
AF = mybir.ActivationFunctionType
NPBF16 = ml_dtypes.bfloat16

NEFF_DUMP = "/tmp/lstm_kernel.neff"

# gate order in the packed weight/bias layout — o LAST so the post-matmul
# tail after the final o-matmul is only: o-ACT, h-mul, h-DMA (c's chain
# depends on f/i/g and completes while o matmuls still run)
#   slot 0: f (sigmoid), 1: i (sigmoid), 2: g (tanh), 3: o (sigmoid)


def build_module(b_core=B_CORE, n_cores=N_CORES):
    nc = bacc.Bacc(
        "TRN2",
        target_bir_lowering=False,
        debug=False,
        num_devices=n_cores,
    )
    # xs[k, pair, p, b]: pair 0 = xT chunk k, pair 1 = sT chunk k (bf16)
    xs = nc.dram_tensor("xs", [4, 2, 128, b_core], BF16, kind="ExternalInput").ap()
    # wt[p, k*2048 + jc*128 + m] = W_gate[jslot*128+m, k*128+p], jc = gate*4+jslot
    wt = nc.dram_tensor("wt", [128, 8192], BF16, kind="ExternalInput").ap()
    # bias[p, jc] = b_gate[jslot*128 + p]
    bias = nc.dram_tensor("bias", [128, 16], F32, kind="ExternalInput").ap()
    # out[ch, js, p, b]: ch 0 = c, 1 = h; h_dim = js*128 + p (host undoes)
    out = nc.dram_tensor("out", [2, 4, 128, b_core], BF16, kind="ExternalOutput").ap()

    with tile.TileContext(nc) as tc:
        with (
            tc.tile_pool(name="const", bufs=1) as cpool,
            tc.tile_pool(name="inp", bufs=8) as ipool,
            tc.tile_pool(name="zp", bufs=2) as zpool,
            tc.tile_pool(name="work", bufs=2) as wpool,
            tc.tile_pool(name="ps", bufs=8, space="PSUM") as pspool,
        ):
            # PE warm-up: a throwaway matmul group on memset data starts the
            # p-state clock ramp while the real input/weight DMAs stream in
            junk = cpool.tile([128, NB], BF16)
            nc.gpsimd.memset(junk[:], 0.0)
            warm_ps = pspool.tile([128, NB], F32, tag="ps", name="warm_ps")
            for i in range(14):
                nc.tensor.matmul(
                    warm_ps[:], junk[:, 0:128], junk[:], start=(i == 0), stop=(i == 13)
                )
            junk2 = cpool.tile([128, NB], BF16)
            nc.vector.tensor_copy(junk2[:], warm_ps[:])

            bias_sb = cpool.tile([128, 16], F32)
            nc.gpsimd.dma_start(out=bias_sb[:], in_=bias[:])
            # weights: one tile per k-chunk (readiness per chunk), each
            # half-loaded on the gpsimd + scalar queues in k order so chunk
            # k lands at ~(k+1)*1.3us instead of the whole 2MB at ~8us
            wt_k = []
            for k in range(4):
                wtk = cpool.tile([128, 2048], BF16, tag=f"wt{k}", name=f"wt{k}")
                nc.gpsimd.dma_start(
                    out=wtk[:, 0:1024], in_=wt[:, k * 2048 : k * 2048 + 1024]
                )
                nc.scalar.dma_start(
                    out=wtk[:, 1024:2048], in_=wt[:, k * 2048 + 1024 : (k + 1) * 2048]
                )
                wt_k.append(wtk)

            for blk in range(N_BLOCKS):
                cols = slice(blk * NB, (blk + 1) * NB)
                z = []
                for k in range(4):
                    # one DMA per k: [pair, p, b] -> SBUF [p, pair*NB + b]
                    xs_t = ipool.tile([128, 2 * NB], BF16, tag="xs")
                    nc.sync.dma_start(
                        out=xs_t[:], in_=xs[k, :, :, cols].transpose([1, 0, 2])
                    )
                    z_t = zpool.tile([128, NB], BF16, tag=f"z{k}")
                    nc.vector.tensor_add(z_t[:], xs_t[:, 0:NB], xs_t[:, NB : 2 * NB])
                    z.append(z_t)

                # 4 gate tiles, each [128, 4*NB]: slot jslot at cols
                # [jslot*NB:(jslot+1)*NB] holds j = jslot*128 + p
                gt = [
                    wpool.tile([128, 4 * NB], BF16, tag=f"g{g}", name=f"gate{g}")
                    for g in range(4)
                ]
                funcs = [AF.Sigmoid, AF.Sigmoid, AF.Tanh, AF.Sigmoid]
                f_t, i_t, g_t, o_t = gt

                def gate_act(g, js, ps):
                    jc = g * 4 + js
                    nc.scalar.activation(
                        gt[g][:, js * NB : (js + 1) * NB],
                        ps[:],
                        funcs[g],
                        bias=bias_sb[:, jc : jc + 1],
                    )

                def gate_group(g, js):
                    jc = g * 4 + js
                    ps = pspool.tile([128, NB], F32, tag="ps", name="ps")
                    for k in range(4):
                        nc.tensor.matmul(
                            ps[:],
                            wt_k[k][:, jc * 128 : (jc + 1) * 128],
                            z[k][:],
                            start=(k == 0),
                            stop=(k == 3),
                        )
                    gate_act(g, js, ps)

                if blk == 0:
                    # k-major over the first 8 groups (f, i gates): each
                    # weight chunk k feeds 8 matmuls as soon as it lands,
                    # so the PE streams while later chunks are in flight
                    ps8 = [
                        pspool.tile([128, NB], F32, tag="ps", name=f"ps_km{j}")
                        for j in range(8)
                    ]
                    for k in range(4):
                        for jc in range(8):
                            nc.tensor.matmul(
                                ps8[jc][:],
                                wt_k[k][:, jc * 128 : (jc + 1) * 128],
                                z[k][:],
                                start=(k == 0),
                                stop=(k == 3),
                            )
                    for jc in range(8):
                        gate_act(jc // 4, jc % 4, ps8[jc])
                    for js in range(4):  # g gate
                        gate_group(2, js)
                else:
                    for g in range(3):  # f, i, g gates
                        for js in range(4):
                            gate_group(g, js)

                # c-chain in two js-halves: each half starts as soon as its
                # i/g slots are activated, overlapping the o-gate matmuls
                prod = wpool.tile([128, 4 * NB], BF16, tag="prod")
                c_t = wpool.tile([128, 4 * NB], BF16, tag="c")
                tc_t = wpool.tile([128, 4 * NB], BF16, tag="tc")
                for half in range(2):
                    hcols = slice(half * 2 * NB, (half + 1) * 2 * NB)
                    nc.vector.tensor_mul(prod[:, hcols], i_t[:, hcols], g_t[:, hcols])
                    nc.vector.tensor_add(c_t[:, hcols], f_t[:, hcols], prod[:, hcols])
                    nc.scalar.activation(tc_t[:, hcols], c_t[:, hcols], AF.Tanh)
                # all of c in one transposed-AP DMA: SBUF (p, js*NB+b) ->
                # DRAM out[0, js, p, b]
                nc.gpsimd.dma_start(
                    out=out[0, :, :, cols].transpose([1, 0, 2]), in_=c_t[:]
                )

                # o gate + per-slot h so the post-matmul tail is one slot
                # deep; h on the sync queue (idle at the block tail)
                h_t = wpool.tile([128, 4 * NB], BF16, tag="h")
                for js in range(4):
                    gate_group(3, js)
                    bcols = slice(js * NB, (js + 1) * NB)
                    nc.vector.tensor_mul(h_t[:, bcols], tc_t[:, bcols], o_t[:, bcols])
                    nc.sync.dma_start(out=out[1, js, :, cols], in_=h_t[:, bcols])

    nc.compile()
    return nc


def pack_inputs(inputs, short_term_memory, Wf, bf, Wi, bi, Wg, bg, Wo, bo):
    x = np.asarray(inputs, np.float32).astype(NPBF16)
    s = np.asarray(short_term_memory, np.float32).astype(NPBF16)
    # xs[core, k, pair, p, b] = (x if pair==0 else s).T chunk:
    #   [B, H] -> [n_cores, B_CORE, 4, 128] -> transpose -> [n_cores, 4, 128, B_CORE]
    xT = x.reshape(N_CORES, B_CORE, 4, 128).transpose(0, 2, 3, 1)
    sT = s.reshape(N_CORES, B_CORE, 4, 128).transpose(0, 2, 3, 1)
    xs = np.ascontiguousarray(np.stack([xT, sT], axis=2)).reshape(
        N_CORES * 4, 2, 128, B_CORE
    )

    Ws = [Wf, Wi, Wg, Wo]
    bs = [bf, bi, bg, bo]
    wt = np.empty((128, 8192), NPBF16)
    for k in range(4):
        for g, W in enumerate(Ws):
            # columns [k*2048 + g*512 : +512] = W.T[k*128:(k+1)*128, :]
            wt[:, k * 2048 + g * 512 : k * 2048 + (g + 1) * 512] = (
                np.asarray(W, np.float32).T[k * 128 : (k + 1) * 128, :].astype(NPBF16)
            )
    bias = np.empty((128, 16), np.float32)
    for g, b in enumerate(bs):
        bias[:, g * 4 : (g + 1) * 4] = np.asarray(b, np.float32).reshape(4, 128).T
    return {"xs": xs, "wt": wt, "bias": bias}


class Runner:
    """Compiles the module once and keeps a reusable jitted executor."""

    def __init__(self, nc=None, n_cores=N_CORES):
        import jax
        from concourse import bass2jax as b2j

        self.jax = jax
        self.n_cores = n_cores
        self.nc = nc or build_module(n_cores=n_cores)
        b2j.install_neuronx_cc_hook()

        # dump the final (renamed) NEFF so neuron-profile can pair it with NTFFs
        if not getattr(b2j, "_neff_dump_patched", False):
            orig = b2j.rename_neff_tensors_and_patch_header

            def _patched(neff_path, mapping):
                data = orig(neff_path, mapping)
                with open(NEFF_DUMP, "wb") as f:
                    f.write(data)
                return data

            b2j.rename_neff_tensors_and_patch_header = _patched
            b2j._neff_dump_patched = True

        from jax.experimental.shard_map import shard_map
        from jax.sharding import Mesh, NamedSharding, PartitionSpec

        part_name = (
            self.nc.partition_id_tensor.name if self.nc.partition_id_tensor else None
        )
        in_names, out_names, out_avals = [], [], []
        self.out_shapes = {}
        for alloc in self.nc.m.functions[0].allocations:
            if not isinstance(alloc, mybir.MemoryLocationSet):
                continue
            name = alloc.memorylocations[0].name
            if alloc.kind == "ExternalInput":
                if name != part_name:
                    in_names.append(name)
            elif alloc.kind == "ExternalOutput":
                out_names.append(name)
                shape = tuple(alloc.tensor_shape)
                dt = mybir.dt.np(alloc.dtype)
                out_avals.append(jax.core.ShapedArray(shape, dt))
                self.out_shapes[name] = (shape, dt)
        self.in_names, self.out_names = in_names, out_names
        nc_ref = self.nc

        bind_names = list(in_names) + list(out_names)
        if part_name is not None:
            bind_names.append(part_name)

        def _body(*args):
            operands = list(args)
            if part_name is not None:
                operands.append(b2j.partition_id_tensor())
            outs = b2j._bass_exec_p.bind(
                *operands,
                out_avals=tuple(out_avals),
                in_names=tuple(bind_names),
                out_names=tuple(out_names),
                lowering_input_output_aliases=(),
                sim_require_finite=False,
                sim_require_nnan=False,
                nc=nc_ref,
            )
            return tuple(outs)

        devices = jax.devices()[: self.n_cores]
        mesh = Mesh(np.asarray(devices), ("core",))
        spec = PartitionSpec("core")
        n_args = len(in_names) + len(out_names)
        self.sharding = NamedSharding(mesh, spec)
        self.fn = jax.jit(
            shard_map(
                _body,
                mesh=mesh,
                in_specs=(spec,) * n_args,
                out_specs=(spec,) * len(out_names),
                check_rep=False,
            ),
            keep_unused=True,
        )
        self._dev_args = None

    def stage(self, packed):
        """Transfer inputs (sharded/replicated as needed) to devices once."""
        jax = self.jax
        nc_n = self.n_cores
        args = []
        for name in self.in_names:
            a = packed[name]
            if name == "xs":
                glob = a  # already [n_cores*4, 2, 128, B_CORE]; shard axis 0
            else:
                glob = np.concatenate([a] * nc_n, axis=0)  # replicate
            args.append(glob)
        for name in self.out_names:
            shape, dt = self.out_shapes[name]
            args.append(np.zeros((shape[0] * nc_n,) + shape[1:], dt))
        self._dev_args = [jax.device_put(a, self.sharding) for a in args]

    def execute(self):
        outs = self.fn(*self._dev_args)
        self.jax.block_until_ready(outs)
        return outs

    def run(self, packed):
        self.stage(packed)
        outs = self.execute()
        res = {}
        for name, arr in zip(self.out_names, outs):
            a = np.asarray(arr)  # [n_cores*d0, ...]
            shape, _ = self.out_shapes[name]
            res[name] = a.reshape((self.n_cores, shape[0]) + tuple(shape[1:]))
        return res


_RUNNER = None


def _get_runner():
    global _RUNNER
    if _RUNNER is None:
        _RUNNER = Runner()
    return _RUNNER


def kernel(**inputs):
    r = _get_runner()
    packed = pack_inputs(**inputs)
    res = r.run(packed)
    per_core = res["out"]  # [8, 2, 4, 128, B_CORE] bf16: (core, ch, js, p, b)
    full = per_core.transpose(1, 0, 4, 2, 3).reshape(2, B, H)
    return np.ascontiguousarray(full).astype(np.float32)


if __name__ == "__main__":
    nc = build_module()
    print("module built + compiled OK")


# revision 24
# speedup vs baseline: 1.0193x; 1.0193x over previous
"""LSTMCell (B=65536, H=512) Bass/Tile kernel for 8 trn2 NeuronCores.

Data-parallel over batch: each core processes 8192 rows, as 16 blocks
of 512 batch columns in a TRANSPOSED [feature, batch] layout:

  host: xT/sT staged as bf16 [512, 8192] per core (batch transposed),
        weights packed so each matmul lhsT is a [128h, 128j] W chunk.
  per 512-batch block:
    zT = xT + sT                   (DVE, 4x [128,512] bf16)
    for each of 16 j-chunks (4 gates x 4 slots):
      psum[128j, 512b] = sum_k W_chunk.T @ zT_chunk   (PE, bf16)
      act = sigmoid/tanh(psum + bias_j)  (ACT, per-partition bias fused)
    prod = i*g; c = f + prod; tc = tanh(c); h = tc*o   (DVE + ACT, [128,2048])
    DMA out c,h as bf16 in [h_dim, batch] layout; host transposes back.

vs the previous version this removes the PE transposes, removes the
[128,2048] fp32 DVE bias-add (the old bottleneck: DVE was 76% busy),
and halves DMA traffic (bf16 I/O).
"""

import os
import sys

if "/opt/trn_rl_repo" not in sys.path:
    sys.path.insert(0, "/opt/trn_rl_repo")

import ml_dtypes
import numpy as np

import concourse.bacc as bacc
import concourse.mybir as mybir
import concourse.tile as tile

N_CORES = 8
B, H = 65536, 512
B_CORE = B // N_CORES  # 8192
NB = 512  # batch columns per block
N_BLOCKS = B_CORE // NB  # 16
F32 = mybir.dt.float32
BF16 = mybir.dt.bfloat16
AF = mybir.ActivationFunctionType
NPBF16 = ml_dtypes.bfloat16

NEFF_DUMP = "/tmp/lstm_kernel.neff"

# gate order in the packed weight/bias layout — o LAST so the post-matmul
# tail after the final o-matmul is only: o-ACT, h-mul, h-DMA (c's chain
# depends on f/i/g and completes while o matmuls still run)
#   slot 0: f (sigmoid), 1: i (sigmoid), 2: g (tanh), 3: o (sigmoid)


def build_module(b_core=B_CORE, n_cores=N_CORES):
    nc = bacc.Bacc(
        "TRN2",
        target_bir_lowering=False,
        debug=False,
        num_devices=n_cores,
    )
    # xs[k, pair, p, b]: pair 0 = xT chunk k, pair 1 = sT chunk k (bf16)
    xs = nc.dram_tensor("xs", [4, 2, 128, b_core], BF16, kind="ExternalInput").ap()
    # wt[p, k*2048 + jc*128 + m] = W_gate[jslot*128+m, k*128+p], jc = gate*4+jslot
    wt = nc.dram_tensor("wt", [128, 8192], BF16, kind="ExternalInput").ap()
    # bias[p, jc] = b_gate[jslot*128 + p]
    bias = nc.dram_tensor("bias", [128, 16], F32, kind="ExternalInput").ap()
    # out[ch, js, p, b]: ch 0 = c, 1 = h; h_dim = js*128 + p (host undoes)
    out = nc.dram_tensor("out", [2, 4, 128, b_core], BF16, kind="ExternalOutput").ap()

    with tile.TileContext(nc) as tc:
        with (
            tc.tile_pool(name="const", bufs=1) as cpool,
            tc.tile_pool(name="inp", bufs=8) as ipool,
            tc.tile_pool(name="zp", bufs=2) as zpool,
            tc.tile_pool(name="work", bufs=2) as wpool,
            tc.tile_pool(name="ps", bufs=8, space="PSUM") as pspool,
        ):
            # PE warm-up: a throwaway matmul group on memset data starts the
            # p-state clock ramp while the real input/weight DMAs stream in
            junk = cpool.tile([128, NB], BF16)
            nc.gpsimd.memset(junk[:], 0.0)
            warm_ps = pspool.tile([128, NB], F32, tag="ps", name="warm_ps")
            for i in range(6):
                nc.tensor.matmul(
                    warm_ps[:], junk[:, 0:128], junk[:], start=(i == 0), stop=(i == 5)
                )
            junk2 = cpool.tile([128, NB], BF16)
            nc.vector.tensor_copy(junk2[:], warm_ps[:])

            bias_sb = cpool.tile([128, 16], F32)
            nc.gpsimd.dma_start(out=bias_sb[:], in_=bias[:])
            # weights: one tile per k-chunk (readiness per chunk), each
            # half-loaded on the gpsimd + scalar queues in k order so chunk
            # k lands at ~(k+1)*1.3us instead of the whole 2MB at ~8us
            wt_k = []
            for k in range(4):
                wtk = cpool.tile([128, 2048], BF16, tag=f"wt{k}", name=f"wt{k}")
                nc.gpsimd.dma_start(
                    out=wtk[:, 0:1024], in_=wt[:, k * 2048 : k * 2048 + 1024]
                )
                nc.scalar.dma_start(
                    out=wtk[:, 1024:2048], in_=wt[:, k * 2048 + 1024 : (k + 1) * 2048]
                )
                wt_k.append(wtk)

            for blk in range(N_BLOCKS):
                cols = slice(blk * NB, (blk + 1) * NB)
                z = []
                for k in range(4):
                    # one DMA per k: [pair, p, b] -> SBUF [p, pair*NB + b]
                    xs_t = ipool.tile([128, 2 * NB], BF16, tag="xs")
                    nc.sync.dma_start(
                        out=xs_t[:], in_=xs[k, :, :, cols].transpose([1, 0, 2])
                    )
                    z_t = zpool.tile([128, NB], BF16, tag=f"z{k}")
                    nc.vector.tensor_add(z_t[:], xs_t[:, 0:NB], xs_t[:, NB : 2 * NB])
                    z.append(z_t)

                # 4 gate tiles, each [128, 4*NB]: slot jslot at cols
                # [jslot*NB:(jslot+1)*NB] holds j = jslot*128 + p
                gt = [
                    wpool.tile([128, 4 * NB], BF16, tag=f"g{g}", name=f"gate{g}")
                    for g in range(4)
                ]
                funcs = [AF.Sigmoid, AF.Sigmoid, AF.Tanh, AF.Sigmoid]
                f_t, i_t, g_t, o_t = gt

                def gate_act(g, js, ps):
                    jc = g * 4 + js
                    nc.scalar.activation(
                        gt[g][:, js * NB : (js + 1) * NB],
                        ps[:],
                        funcs[g],
                        bias=bias_sb[:, jc : jc + 1],
                    )

                def gate_group(g, js):
                    jc = g * 4 + js
                    ps = pspool.tile([128, NB], F32, tag="ps", name="ps")
                    for k in range(4):
                        nc.tensor.matmul(
                            ps[:],
                            wt_k[k][:, jc * 128 : (jc + 1) * 128],
                            z[k][:],
                            start=(k == 0),
                            stop=(k == 3),
                        )
                    gate_act(g, js, ps)

                if blk == 0:
                    # k-major over the first 8 groups (f, i gates): each
                    # weight chunk k feeds 8 matmuls as soon as it lands,
                    # so the PE streams while later chunks are in flight
                    ps8 = [
                        pspool.tile([128, NB], F32, tag="ps", name=f"ps_km{j}")
                        for j in range(8)
                    ]
                    for k in range(4):
                        for jc in range(8):
                            nc.tensor.matmul(
                                ps8[jc][:],
                                wt_k[k][:, jc * 128 : (jc + 1) * 128],
                                z[k][:],
                                start=(k == 0),
                                stop=(k == 3),
                            )
                    for jc in range(8):
                        gate_act(jc // 4, jc % 4, ps8[jc])
                    for js in range(4):  # g gate
                        gate_group(2, js)
                else:
                    for g in range(3):  # f, i, g gates
                        for js in range(4):
                            gate_group(g, js)

                # c-chain in two js-halves: each half starts as soon as its
                # i/g slots are activated, overlapping the o-gate matmuls
                prod = wpool.tile([128, 4 * NB], BF16, tag="prod")
                c_t = wpool.tile([128, 4 * NB], BF16, tag="c")
                tc_t = wpool.tile([128, 4 * NB], BF16, tag="tc")
                for half in range(2):
                    hcols = slice(half * 2 * NB, (half + 1) * 2 * NB)
                    nc.vector.tensor_mul(prod[:, hcols], i_t[:, hcols], g_t[:, hcols])
                    nc.vector.tensor_add(c_t[:, hcols], f_t[:, hcols], prod[:, hcols])
                    nc.scalar.activation(tc_t[:, hcols], c_t[:, hcols], AF.Tanh)
                # all of c in one transposed-AP DMA: SBUF (p, js*NB+b) ->
                # DRAM out[0, js, p, b]
                nc.gpsimd.dma_start(
                    out=out[0, :, :, cols].transpose([1, 0, 2]), in_=c_t[:]
                )

                # o gate + per-slot h so the post-matmul tail is one slot
                # deep; h on the sync queue (idle at the block tail)
                h_t = wpool.tile([128, 4 * NB], BF16, tag="h")
                for js in range(4):
                    gate_group(3, js)
                    bcols = slice(js * NB, (js + 1) * NB)
                    nc.vector.tensor_mul(h_t[:, bcols], tc_t[:, bcols], o_t[:, bcols])
                    nc.sync.dma_start(out=out[1, js, :, cols], in_=h_t[:, bcols])

    nc.compile()
    return nc


def pack_inputs(inputs, short_term_memory, Wf, bf, Wi, bi, Wg, bg, Wo, bo):
    x = np.asarray(inputs, np.float32).astype(NPBF16)
    s = np.asarray(short_term_memory, np.float32).astype(NPBF16)
    # xs[core, k, pair, p, b] = (x if pair==0 else s).T chunk:
    #   [B, H] -> [n_cores, B_CORE, 4, 128] -> transpose -> [n_cores, 4, 128, B_CORE]
    xT = x.reshape(N_CORES, B_CORE, 4, 128).transpose(0, 2, 3, 1)
    sT = s.reshape(N_CORES, B_CORE, 4, 128).transpose(0, 2, 3, 1)
    xs = np.ascontiguousarray(np.stack([xT, sT], axis=2)).reshape(
        N_CORES * 4, 2, 128, B_CORE
    )

    Ws = [Wf, Wi, Wg, Wo]
    bs = [bf, bi, bg, bo]
    wt = np.empty((128, 8192), NPBF16)
    for k in range(4):
        for g, W in enumerate(Ws):
            # columns [k*2048 + g*512 : +512] = W.T[k*128:(k+1)*128, :]
            wt[:, k * 2048 + g * 512 : k * 2048 + (g + 1) * 512] = (
                np.asarray(W, np.float32).T[k * 128 : (k + 1) * 128, :].astype(NPBF16)
            )
    bias = np.empty((128, 16), np.float32)
    for g, b in enumerate(bs):
        bias[:, g * 4 : (g + 1) * 4] = np.asarray(b, np.float32).reshape(4, 128).T
    return {"xs": xs, "wt": wt, "bias": bias}


class Runner:
    """Compiles the module once and keeps a reusable jitted executor."""

    def __init__(self, nc=None, n_cores=N_CORES):
        import jax
        from concourse import bass2jax as b2j

        self.jax = jax
        self.n_cores = n_cores
        self.nc = nc or build_module(n_cores=n_cores)
        b2j.install_neuronx_cc_hook()

        # dump the final (renamed) NEFF so neuron-profile can pair it with NTFFs
        if not getattr(b2j, "_neff_dump_patched", False):
            orig = b2j.rename_neff_tensors_and_patch_header

            def _patched(neff_path, mapping):
                data = orig(neff_path, mapping)
                with open(NEFF_DUMP, "wb") as f:
                    f.write(data)
                return data

            b2j.rename_neff_tensors_and_patch_header = _patched
            b2j._neff_dump_patched = True

        from jax.experimental.shard_map import shard_map
        from jax.sharding import Mesh, NamedSharding, PartitionSpec

        part_name = (
            self.nc.partition_id_tensor.name if self.nc.partition_id_tensor else None
        )
        in_names, out_names, out_avals = [], [], []
        self.out_shapes = {}
        for alloc in self.nc.m.functions[0].allocations:
            if not isinstance(alloc, mybir.MemoryLocationSet):
                continue
            name = alloc.memorylocations[0].name
            if alloc.kind == "ExternalInput":
                if name != part_name:
                    in_names.append(name)
            elif alloc.kind == "ExternalOutput":
                out_names.append(name)
                shape = tuple(alloc.tensor_shape)
                dt = mybir.dt.np(alloc.dtype)
                out_avals.append(jax.core.ShapedArray(shape, dt))
                self.out_shapes[name] = (shape, dt)
        self.in_names, self.out_names = in_names, out_names
        nc_ref = self.nc

        bind_names = list(in_names) + list(out_names)
        if part_name is not None:
            bind_names.append(part_name)

        def _body(*args):
            operands = list(args)
            if part_name is not None:
                operands.append(b2j.partition_id_tensor())
            outs = b2j._bass_exec_p.bind(
                *operands,
                out_avals=tuple(out_avals),
                in_names=tuple(bind_names),
                out_names=tuple(out_names),
                lowering_input_output_aliases=(),
                sim_require_finite=False,
                sim_require_nnan=False,
                nc=nc_ref,
            )
            return tuple(outs)

        devices = jax.devices()[: self.n_cores]
        mesh = Mesh(np.asarray(devices), ("core",))
        spec = PartitionSpec("core")
        n_args = len(in_names) + len(out_names)
        self.sharding = NamedSharding(mesh, spec)
        self.fn = jax.jit(
            shard_map(
                _body,
                mesh=mesh,
                in_specs=(spec,) * n_args,
                out_specs=(spec,) * len(out_names),
                check_rep=False,
            ),
            keep_unused=True,
        )
        self._dev_args = None

    def stage(self, packed):
        """Transfer inputs (sharded/replicated as needed) to devices once."""
        jax = self.jax
        nc_n = self.n_cores
        args = []
        for name in self.in_names:
            a = packed[name]
            if name == "xs":
                glob = a  # already [n_cores*4, 2, 128, B_CORE]; shard axis 0
            else:
                glob = np.concatenate([a] * nc_n, axis=0)  # replicate
            args.append(glob)
        for name in self.out_names:
            shape, dt = self.out_shapes[name]
            args.append(np.zeros((shape[0] * nc_n,) + shape[1:], dt))
        self._dev_args = [jax.device_put(a, self.sharding) for a in args]

    def execute(self):
        outs = self.fn(*self._dev_args)
        self.jax.block_until_ready(outs)
        return outs

    def run(self, packed):
        self.stage(packed)
        outs = self.execute()
        res = {}
        for name, arr in zip(self.out_names, outs):
            a = np.asarray(arr)  # [n_cores*d0, ...]
            shape, _ = self.out_shapes[name]
            res[name] = a.reshape((self.n_cores, shape[0]) + tuple(shape[1:]))
        return res


_RUNNER = None


def _get_runner():
    global _RUNNER
    if _RUNNER is None:
        _RUNNER = Runner()
    return _RUNNER


def kernel(**inputs):
    r = _get_runner()
    packed = pack_inputs(**inputs)
    res = r.run(packed)
    per_core = res["out"]  # [8, 2, 4, 128, B_CORE] bf16: (core, ch, js, p, b)
    full = per_core.transpose(1, 0, 4, 2, 3).reshape(2, B, H)
    return np.ascontiguousarray(full).astype(np.float32)


if __name__ == "__main__":
    nc = build_module()
    print("module built + compiled OK")


# revision 25
# speedup vs baseline: 1.0254x; 1.0060x over previous
"""LSTMCell (B=65536, H=512) Bass/Tile kernel for 8 trn2 NeuronCores.

Data-parallel over batch: each core processes 8192 rows, as 16 blocks
of 512 batch columns in a TRANSPOSED [feature, batch] layout:

  host: xT/sT staged as bf16 [512, 8192] per core (batch transposed),
        weights packed so each matmul lhsT is a [128h, 128j] W chunk.
  per 512-batch block:
    zT = xT + sT                   (DVE, 4x [128,512] bf16)
    for each of 16 j-chunks (4 gates x 4 slots):
      psum[128j, 512b] = sum_k W_chunk.T @ zT_chunk   (PE, bf16)
      act = sigmoid/tanh(psum + bias_j)  (ACT, per-partition bias fused)
    prod = i*g; c = f + prod; tc = tanh(c); h = tc*o   (DVE + ACT, [128,2048])
    DMA out c,h as bf16 in [h_dim, batch] layout; host transposes back.

vs the previous version this removes the PE transposes, removes the
[128,2048] fp32 DVE bias-add (the old bottleneck: DVE was 76% busy),
and halves DMA traffic (bf16 I/O).
"""

import os
import sys

if "/opt/trn_rl_repo" not in sys.path:
    sys.path.insert(0, "/opt/trn_rl_repo")

import ml_dtypes
import numpy as np

import concourse.bacc as bacc
import concourse.mybir as mybir
import concourse.tile as tile

N_CORES = 8
B, H = 65536, 512
B_CORE = B // N_CORES  # 8192
NB = 512  # batch columns per block
N_BLOCKS = B_CORE // NB  # 16
F32 = mybir.dt.float32
BF16 = mybir.dt.bfloat16
AF = mybir.ActivationFunctionType
NPBF16 = ml_dtypes.bfloat16

NEFF_DUMP = "/tmp/lstm_kernel.neff"

# gate order in the packed weight/bias layout — o LAST so the post-matmul
# tail after the final o-matmul is only: o-ACT, h-mul, h-DMA (c's chain
# depends on f/i/g and completes while o matmuls still run)
#   slot 0: f (sigmoid), 1: i (sigmoid), 2: g (tanh), 3: o (sigmoid)


def build_module(b_core=B_CORE, n_cores=N_CORES):
    nc = bacc.Bacc(
        "TRN2",
        target_bir_lowering=False,
        debug=False,
        num_devices=n_cores,
    )
    # xs[k, pair, p, b]: pair 0 = xT chunk k, pair 1 = sT chunk k (bf16)
    xs = nc.dram_tensor("xs", [4, 2, 128, b_core], BF16, kind="ExternalInput").ap()
    # wt[p, k*2048 + jc*128 + m] = W_gate[jslot*128+m, k*128+p], jc = gate*4+jslot
    wt = nc.dram_tensor("wt", [128, 8192], BF16, kind="ExternalInput").ap()
    # bias[p, jc] = b_gate[jslot*128 + p]
    bias = nc.dram_tensor("bias", [128, 16], F32, kind="ExternalInput").ap()
    # out[ch, js, p, b]: ch 0 = c, 1 = h; h_dim = js*128 + p (host undoes)
    out = nc.dram_tensor("out", [2, 4, 128, b_core], BF16, kind="ExternalOutput").ap()

    with tile.TileContext(nc) as tc:
        with (
            tc.tile_pool(name="const", bufs=1) as cpool,
            tc.tile_pool(name="inp", bufs=8) as ipool,
            tc.tile_pool(name="zp", bufs=2) as zpool,
            tc.tile_pool(name="work", bufs=2) as wpool,
            tc.tile_pool(name="ps", bufs=8, space="PSUM") as pspool,
        ):
            # weights: one tile per k-chunk (readiness per chunk). wt0 is
            # the block-0 k-major critical path: its four 256KB quarters go
            # FIRST on both DMA queues (per-DMA completion is ~issue +
            # 4.5us fixed + transfer, so queue position dominates)
            wt_k = [
                cpool.tile([128, 2048], BF16, tag=f"wt{k}", name=f"wt{k}")
                for k in range(4)
            ]
            for q in range(2):
                nc.gpsimd.dma_start(
                    out=wt_k[0][:, q * 512 : (q + 1) * 512],
                    in_=wt[:, q * 512 : (q + 1) * 512],
                )
                nc.scalar.dma_start(
                    out=wt_k[0][:, 1024 + q * 512 : 1024 + (q + 1) * 512],
                    in_=wt[:, 1024 + q * 512 : 1024 + (q + 1) * 512],
                )
            bias_sb = cpool.tile([128, 16], F32)
            nc.gpsimd.dma_start(out=bias_sb[:], in_=bias[:])

            # PE warm-up: a throwaway matmul group on memset data runs the
            # p-state clock ramp while the first DMA completions are pending
            junk = cpool.tile([128, NB], BF16)
            nc.gpsimd.memset(junk[:], 0.0)
            warm_ps = pspool.tile([128, NB], F32, tag="ps", name="warm_ps")
            for i in range(6):
                nc.tensor.matmul(
                    warm_ps[:], junk[:, 0:128], junk[:], start=(i == 0), stop=(i == 5)
                )
            junk2 = cpool.tile([128, NB], BF16)
            nc.vector.tensor_copy(junk2[:], warm_ps[:])

            for k in range(1, 4):
                nc.gpsimd.dma_start(
                    out=wt_k[k][:, 0:1024], in_=wt[:, k * 2048 : k * 2048 + 1024]
                )
                nc.scalar.dma_start(
                    out=wt_k[k][:, 1024:2048],
                    in_=wt[:, k * 2048 + 1024 : (k + 1) * 2048],
                )

            for blk in range(N_BLOCKS):
                cols = slice(blk * NB, (blk + 1) * NB)
                z = []
                for k in range(4):
                    # one DMA per k: [pair, p, b] -> SBUF [p, pair*NB + b]
                    xs_t = ipool.tile([128, 2 * NB], BF16, tag="xs")
                    nc.sync.dma_start(
                        out=xs_t[:], in_=xs[k, :, :, cols].transpose([1, 0, 2])
                    )
                    z_t = zpool.tile([128, NB], BF16, tag=f"z{k}")
                    nc.vector.tensor_add(z_t[:], xs_t[:, 0:NB], xs_t[:, NB : 2 * NB])
                    z.append(z_t)

                # 4 gate tiles, each [128, 4*NB]: slot jslot at cols
                # [jslot*NB:(jslot+1)*NB] holds j = jslot*128 + p
                gt = [
                    wpool.tile([128, 4 * NB], BF16, tag=f"g{g}", name=f"gate{g}")
                    for g in range(4)
                ]
                funcs = [AF.Sigmoid, AF.Sigmoid, AF.Tanh, AF.Sigmoid]
                f_t, i_t, g_t, o_t = gt

                def gate_act(g, js, ps):
                    jc = g * 4 + js
                    nc.scalar.activation(
                        gt[g][:, js * NB : (js + 1) * NB],
                        ps[:],
                        funcs[g],
                        bias=bias_sb[:, jc : jc + 1],
                    )

                def gate_group(g, js):
                    jc = g * 4 + js
                    ps = pspool.tile([128, NB], F32, tag="ps", name="ps")
                    for k in range(4):
                        nc.tensor.matmul(
                            ps[:],
                            wt_k[k][:, jc * 128 : (jc + 1) * 128],
                            z[k][:],
                            start=(k == 0),
                            stop=(k == 3),
                        )
                    gate_act(g, js, ps)

                if blk == 0:
                    # k-major over the first 8 groups (f, i gates): each
                    # weight chunk k feeds 8 matmuls as soon as it lands,
                    # so the PE streams while later chunks are in flight
                    ps8 = [
                        pspool.tile([128, NB], F32, tag="ps", name=f"ps_km{j}")
                        for j in range(8)
                    ]
                    for k in range(4):
                        for jc in range(8):
                            nc.tensor.matmul(
                                ps8[jc][:],
                                wt_k[k][:, jc * 128 : (jc + 1) * 128],
                                z[k][:],
                                start=(k == 0),
                                stop=(k == 3),
                            )
                    for jc in range(8):
                        gate_act(jc // 4, jc % 4, ps8[jc])
                    for js in range(4):  # g gate
                        gate_group(2, js)
                else:
                    for g in range(3):  # f, i, g gates
                        for js in range(4):
                            gate_group(g, js)

                # c-chain in two js-halves: each half starts as soon as its
                # i/g slots are activated, overlapping the o-gate matmuls
                prod = wpool.tile([128, 4 * NB], BF16, tag="prod")
                c_t = wpool.tile([128, 4 * NB], BF16, tag="c")
                tc_t = wpool.tile([128, 4 * NB], BF16, tag="tc")
                for half in range(2):
                    hcols = slice(half * 2 * NB, (half + 1) * 2 * NB)
                    nc.vector.tensor_mul(prod[:, hcols], i_t[:, hcols], g_t[:, hcols])
                    nc.vector.tensor_add(c_t[:, hcols], f_t[:, hcols], prod[:, hcols])
                    nc.scalar.activation(tc_t[:, hcols], c_t[:, hcols], AF.Tanh)
                # all of c in one transposed-AP DMA: SBUF (p, js*NB+b) ->
                # DRAM out[0, js, p, b]
                nc.gpsimd.dma_start(
                    out=out[0, :, :, cols].transpose([1, 0, 2]), in_=c_t[:]
                )

                # o gate + per-slot h so the post-matmul tail is one slot
                # deep; h on the sync queue (idle at the block tail)
                h_t = wpool.tile([128, 4 * NB], BF16, tag="h")
                for js in range(4):
                    gate_group(3, js)
                    bcols = slice(js * NB, (js + 1) * NB)
                    nc.vector.tensor_mul(h_t[:, bcols], tc_t[:, bcols], o_t[:, bcols])
                    nc.sync.dma_start(out=out[1, js, :, cols], in_=h_t[:, bcols])

    nc.compile()
    return nc


def pack_inputs(inputs, short_term_memory, Wf, bf, Wi, bi, Wg, bg, Wo, bo):
    x = np.asarray(inputs, np.float32).astype(NPBF16)
    s = np.asarray(short_term_memory, np.float32).astype(NPBF16)
    # xs[core, k, pair, p, b] = (x if pair==0 else s).T chunk:
    #   [B, H] -> [n_cores, B_CORE, 4, 128] -> transpose -> [n_cores, 4, 128, B_CORE]
    xT = x.reshape(N_CORES, B_CORE, 4, 128).transpose(0, 2, 3, 1)
    sT = s.reshape(N_CORES, B_CORE, 4, 128).transpose(0, 2, 3, 1)
    xs = np.ascontiguousarray(np.stack([xT, sT], axis=2)).reshape(
        N_CORES * 4, 2, 128, B_CORE
    )

    Ws = [Wf, Wi, Wg, Wo]
    bs = [bf, bi, bg, bo]
    wt = np.empty((128, 8192), NPBF16)
    for k in range(4):
        for g, W in enumerate(Ws):
            # columns [k*2048 + g*512 : +512] = W.T[k*128:(k+1)*128, :]
            wt[:, k * 2048 + g * 512 : k * 2048 + (g + 1) * 512] = (
                np.asarray(W, np.float32).T[k * 128 : (k + 1) * 128, :].astype(NPBF16)
            )
    bias = np.empty((128, 16), np.float32)
    for g, b in enumerate(bs):
        bias[:, g * 4 : (g + 1) * 4] = np.asarray(b, np.float32).reshape(4, 128).T
    return {"xs": xs, "wt": wt, "bias": bias}


class Runner:
    """Compiles the module once and keeps a reusable jitted executor."""

    def __init__(self, nc=None, n_cores=N_CORES):
        import jax
        from concourse import bass2jax as b2j

        self.jax = jax
        self.n_cores = n_cores
        self.nc = nc or build_module(n_cores=n_cores)
        b2j.install_neuronx_cc_hook()

        # dump the final (renamed) NEFF so neuron-profile can pair it with NTFFs
        if not getattr(b2j, "_neff_dump_patched", False):
            orig = b2j.rename_neff_tensors_and_patch_header

            def _patched(neff_path, mapping):
                data = orig(neff_path, mapping)
                with open(NEFF_DUMP, "wb") as f:
                    f.write(data)
                return data

            b2j.rename_neff_tensors_and_patch_header = _patched
            b2j._neff_dump_patched = True

        from jax.experimental.shard_map import shard_map
        from jax.sharding import Mesh, NamedSharding, PartitionSpec

        part_name = (
            self.nc.partition_id_tensor.name if self.nc.partition_id_tensor else None
        )
        in_names, out_names, out_avals = [], [], []
        self.out_shapes = {}
        for alloc in self.nc.m.functions[0].allocations:
            if not isinstance(alloc, mybir.MemoryLocationSet):
                continue
            name = alloc.memorylocations[0].name
            if alloc.kind == "ExternalInput":
                if name != part_name:
                    in_names.append(name)
            elif alloc.kind == "ExternalOutput":
                out_names.append(name)
                shape = tuple(alloc.tensor_shape)
                dt = mybir.dt.np(alloc.dtype)
                out_avals.append(jax.core.ShapedArray(shape, dt))
                self.out_shapes[name] = (shape, dt)
        self.in_names, self.out_names = in_names, out_names
        nc_ref = self.nc

        bind_names = list(in_names) + list(out_names)
        if part_name is not None:
            bind_names.append(part_name)

        def _body(*args):
            operands = list(args)
            if part_name is not None:
                operands.append(b2j.partition_id_tensor())
            outs = b2j._bass_exec_p.bind(
                *operands,
                out_avals=tuple(out_avals),
                in_names=tuple(bind_names),
                out_names=tuple(out_names),
                lowering_input_output_aliases=(),
                sim_require_finite=False,
                sim_require_nnan=False,
                nc=nc_ref,
            )
            return tuple(outs)

        devices = jax.devices()[: self.n_cores]
        mesh = Mesh(np.asarray(devices), ("core",))
        spec = PartitionSpec("core")
        n_args = len(in_names) + len(out_names)
        self.sharding = NamedSharding(mesh, spec)
        self.fn = jax.jit(
            shard_map(
                _body,
                mesh=mesh,
                in_specs=(spec,) * n_args,
                out_specs=(spec,) * len(out_names),
                check_rep=False,
            ),
            keep_unused=True,
        )
        self._dev_args = None

    def stage(self, packed):
        """Transfer inputs (sharded/replicated as needed) to devices once."""
        jax = self.jax
        nc_n = self.n_cores
        args = []
        for name in self.in_names:
            a = packed[name]
            if name == "xs":
                glob = a  # already [n_cores*4, 2, 128, B_CORE]; shard axis 0
            else:
                glob = np.concatenate([a] * nc_n, axis=0)  # replicate
            args.append(glob)
        for name in self.out_names:
            shape, dt = self.out_shapes[name]
            args.append(np.zeros((shape[0] * nc_n,) + shape[1:], dt))
        self._dev_args = [jax.device_put(a, self.sharding) for a in args]

    def execute(self):
        outs = self.fn(*self._dev_args)
        self.jax.block_until_ready(outs)
        return outs

    def run(self, packed):
        self.stage(packed)
        outs = self.execute()
        res = {}
        for name, arr in zip(self.out_names, outs):
            a = np.asarray(arr)  # [n_cores*d0, ...]
            shape, _ = self.out_shapes[name]
            res[name] = a.reshape((self.n_cores, shape[0]) + tuple(shape[1:]))
        return res


_RUNNER = None


def _get_runner():
    global _RUNNER
    if _RUNNER is None:
        _RUNNER = Runner()
    return _RUNNER


def kernel(**inputs):
    r = _get_runner()
    packed = pack_inputs(**inputs)
    res = r.run(packed)
    per_core = res["out"]  # [8, 2, 4, 128, B_CORE] bf16: (core, ch, js, p, b)
    full = per_core.transpose(1, 0, 4, 2, 3).reshape(2, B, H)
    return np.ascontiguousarray(full).astype(np.float32)


if __name__ == "__main__":
    nc = build_module()
    print("module built + compiled OK")
